# revision 46
# baseline (speedup 1.0000x reference)
"""Trainium2 Bass kernel for nn_DynamicShortConvolution.

Reference computation (per token t, channel d):
    h    = silu(x @ w1)                       # [T, H]
    flat = h @ w2 + b2                        # [T, D*W]
    k    = flat.reshape(T, D, W)
    out[t, d] = silu(sum_w k[t, d, w] * x[t - (W-1) + w, d])

Sharding: 8 cores, each one (batch, half-of-T) shard of 2048 tokens plus a
3-token left halo.  All per-core tensors are laid out TRANSPOSED ([D, T],
channels on SBUF partitions) so the conv's token shift is a free-dim offset.

v2: the baseline was elementwise-bound (PSUM evacuation), not matmul-bound.
Per (512-token chunk, d-tile) iteration the 4 conv taps land in two
[128,1024] psum tiles (taps 0+2, taps 1+3) and are drained by three engines
in parallel:
  - ACT pulls taps 0,2 as two [128,512] Identity ops with the per-partition
    b2 bias fused, downcasting to bf16,
  - GPSIMD multiplies those two taps by their x windows (plain
    tensor_tensor mult - GPSIMD cannot read PSUM and has no stt opcode),
  - DVE does f32 stt (bias+mul) for taps 1,3 straight from PSUM,
    one wide [128,1024] bf16 add and the final [128,512] add,
  - ACT runs silu over [128,1024] (two iterations at a time), then DMA ships
    [128,1024] output blocks.
mm1 for chunk c+1 is spread two contraction-tiles per iteration across the
second half of chunk c's dt loop (x for chunk c+1 arrives mid-chunk), into a
dedicated 2-bank psum pool.  Emission is skewed one iteration so no
cross-engine loop (gp -> adds -> silu -> next pull -> gp) serializes.
Input DMA is ordered w1, b2, x[chunk0], w2-by-dtile, x[chunk1..3] so the
first matmuls start after ~3 MB instead of ~13 MB.
"""

import numpy as np

# Problem constants (hardcoded per harness contract).
B, T, D, H, W = 4, 4096, 2048, 256, 4
HALO = W - 1
N_CORES = 8
TOK = (B * T) // N_CORES  # tokens per core = 2048
TCH = 512                 # token chunk (psum bank = 512 fp32)


def _build_nc(tok, d, h, xstride, out_f32=False, gp_stt=True, sim_safe=False):
    """Build the single-core Bass/Tile program.

    tok: tokens per shard; d: channels; h: hidden; xstride: per-dtile column
    stride of the xT sbuf tensor (even, >= tok + HALO).
    gp_stt: route the two bf16 tap multiplies to GPSIMD (else keep them on
    DVE and give GPSIMD the wide add).
    """
    import concourse.bass as bass
    import concourse.bacc as bacc
    import concourse.mybir as mybir
    import concourse.tile as tile

    f32 = mybir.dt.float32
    bf16 = mybir.dt.bfloat16
    AF = mybir.ActivationFunctionType
    ALU = mybir.AluOpType
    # CoreSim has no Silu; Sigmoid exercises the identical dataflow
    AF_ACT = AF.Sigmoid if sim_safe else AF.Silu

    n_dt = d // 128        # d tiles = 16
    n_hc = h // 128        # h tiles = 2
    n_tc = tok // TCH      # token chunks = 4

    nc = bacc.Bacc()

    # DRAM I/O (host-prepared layouts, partition-major so multi-dtile
    # blocks move as ONE dma_start - each dma_start costs ~0.6us of
    # serialized Sync-engine dispatch, so fewer is faster)
    xT = nc.declare_dram_parameter("xT", [128, n_dt, xstride], bf16, isOutput=False)
    w1 = nc.declare_dram_parameter("w1", [128, n_dt, h], bf16, isOutput=False)
    # w2r[hl, hc, dt, w, dl] = w2[hc*128+hl, ((dt*128+dl))*W + w]
    w2r = nc.declare_dram_parameter("w2r", [128, n_hc, n_dt, W, 128], bf16,
                                    isOutput=False)
    # b2r[p, dt*W + w] = b2[(dt*128+p)*W + w]
    b2r = nc.declare_dram_parameter("b2r", [128, n_dt * W], f32, isOutput=False)
    out_dt = f32 if out_f32 else bf16
    # outT[p, (c*n_dt + dt)*TCH + j] = out token c*TCH+j, channel dt*128+p
    outT = nc.declare_dram_parameter("outT", [128, n_tc * n_dt * TCH], out_dt,
                                     isOutput=True)

    with tile.TileContext(nc) as tc:
        with (
            tc.tile_pool(name="resident", bufs=1) as rpool,
            tc.tile_pool(name="work", bufs=3) as wpool,
            tc.tile_pool(name="psum2", bufs=3, space="PSUM") as ppool,
            tc.tile_pool(name="psumH", bufs=2, space="PSUM") as hpool,
        ):
            # ---- resident tiles ----
            xT_sb = rpool.tile([128, n_dt, xstride], bf16, tag="xT")
            w1_sb = rpool.tile([128, n_dt, h], bf16, tag="w1")
            w2_sb = rpool.tile([128, n_hc, n_dt, W, 128], bf16, tag="w2")
            b2_sb = rpool.tile([128, n_dt * W], f32, tag="b2")
            # hT chunk-major: [hc0 512 | hc1 512] per chunk
            hT_sb = rpool.tile([128, n_tc * 2 * TCH], bf16, tag="hT")

            # ---- input DMA, lead-in ordered, 4-dtile blocks ----
            def dma_x_chunk(c):
                a = 0 if c == 0 else c * TCH + HALO
                bnd = c * TCH + TCH + HALO
                for dt in range(0, n_dt, 4):
                    nc.sync.dma_start(
                        xT_sb[:, dt:dt + 4, a:bnd], xT[:, dt:dt + 4, a:bnd])

            def dma_w2(a, b):
                for hc in range(n_hc):
                    nc.sync.dma_start(
                        w2_sb[:, hc, a:b], w2r[:, hc, a:b])

            for dt in range(0, n_dt, 4):
                nc.sync.dma_start(w1_sb[:, dt:dt + 4], w1[:, dt:dt + 4])
            dma_x_chunk(0)
            dma_w2(0, 2)
            nc.sync.dma_start(b2_sb[:], b2r[:])
            dma_w2(2, 4)
            dma_x_chunk(1)
            dma_w2(4, 8)
            dma_x_chunk(2)
            dma_w2(8, 16)
            dma_x_chunk(3)

            def x_slice(dt, col, n):
                return xT_sb[:, dt, col: col + n]

            def b2s(dt, w):
                return b2_sb[:, dt * W + w: dt * W + w + 1]

            def hslice(c, hc):
                return hT_sb[:, c * 1024 + hc * TCH: c * 1024 + (hc + 1) * TCH]

            def mm1_mms(c, hps, q):
                # contraction tile q of mm1 for chunk c
                for hc in range(n_hc):
                    nc.tensor.matmul(
                        hps[hc][:],
                        w1_sb[:, q, hc * 128: hc * 128 + 128],
                        x_slice(q, HALO + c * TCH, TCH),
                        start=(q == 0), stop=(q == n_dt - 1),
                    )

            def mm1_silu(c, hps):
                for hc in range(n_hc):
                    nc.scalar.activation(hslice(c, hc), hps[hc][:], AF_ACT)

            def hps_alloc():
                tiles = []
                for hc in range(n_hc):
                    hpt = hpool.tile([128, TCH], f32, tag="hps", name=f"hps{hc}")
                    tiles.append(hpt)
                return tiles

            # ---- chunk 0 mm1 up front ----
            hps = hps_alloc()
            for q in range(n_dt):
                mm1_mms(0, hps, q)
            mm1_silu(0, hps)

            sw_q = []    # (mbuf, finbuf, half): s-wide + fin, 1-iter skew
            silu_q = []  # (finbuf, git): completed pairs awaiting silu

            add_eng = nc.vector if gp_stt else nc.gpsimd

            def emit_sw():
                while sw_q:
                    mb, fb, half = sw_q.pop(0)
                    sb = wpool.tile([128, 1024], bf16, tag="s")
                    add_eng.tensor_tensor(
                        sb[:], mb[:, :1024], mb[:, 1024:], op=ALU.add)
                    nc.vector.tensor_tensor(
                        fb[:, half * TCH:(half + 1) * TCH],
                        sb[:, :TCH], sb[:, TCH:], op=ALU.add)

            otbuf = [None]

            def emit_silu(drain=False):
                # one [128,8192] output DMA per eight silu pairs (each
                # dma_start costs ~0.6us of serialized Sync dispatch)
                while silu_q:
                    fb, git = silu_q.pop(0)
                    part = (git % 16) // 2
                    if part == 0:
                        otbuf[0] = wpool.tile([128, 8192], out_dt, tag="ot",
                                              name="ot")
                    ot = otbuf[0]
                    nc.scalar.activation(
                        ot[:, part * 1024:(part + 1) * 1024], fb[:], AF_ACT)
                    if part == 7 or drain:
                        nc.sync.dma_start(
                            outT[:, (git - 2 * part) * TCH:
                                 git * TCH + 1024], ot[:, :(part + 1) * 1024])

            finbuf = None
            for c in range(n_tc):
                j0 = c * TCH
                hps_next = hps_alloc() if c + 1 < n_tc else None
                for dt in range(n_dt):
                    it = c * n_dt + dt
                    # tensor: mm2 matmuls - taps 0,2 -> p02; taps 1,3 -> p13
                    p02 = ppool.tile([128, 1024], f32, tag="ps")
                    p13 = ppool.tile([128, 1024], f32, tag="ps")
                    for pt, taps in ((p02, (0, 2)), (p13, (1, 3))):
                        for half, w in enumerate(taps):
                            for hc in range(n_hc):
                                nc.tensor.matmul(
                                    pt[:, half * TCH:(half + 1) * TCH],
                                    w2_sb[:, hc, dt, w],
                                    hslice(c, hc),
                                    start=(hc == 0), stop=(hc == n_hc - 1),
                                )
                    # tensor: two contraction tiles of mm1(c+1) in the back
                    # half of the chunk (x for c+1 lands mid-chunk)
                    if hps_next is not None and dt >= n_dt - n_dt // 2:
                        q = 2 * (dt - n_dt // 2)
                        mm1_mms(c + 1, hps_next, q)
                        mm1_mms(c + 1, hps_next, q + 1)
                    # ACT: pull taps 0,2 to bf16 with the b2 bias fused
                    t02 = wpool.tile([128, 1024], bf16, tag="t02")
                    nc.scalar.activation(t02[:, :TCH], p02[:, :TCH],
                                         AF.Identity, bias=b2s(dt, 0))
                    nc.scalar.activation(t02[:, TCH:], p02[:, TCH:],
                                         AF.Identity, bias=b2s(dt, 2))
                    # GPSIMD (or DVE): multiply pulled taps by x windows
                    mbuf = wpool.tile([128, 2048], bf16, tag="m")
                    mul_eng = nc.gpsimd if gp_stt else nc.vector
                    mul_eng.tensor_tensor(
                        mbuf[:, 0:TCH], t02[:, :TCH],
                        x_slice(dt, j0 + 0, TCH), op=ALU.mult)
                    mul_eng.tensor_tensor(
                        mbuf[:, 1024:1024 + TCH], t02[:, TCH:],
                        x_slice(dt, j0 + 2, TCH), op=ALU.mult)
                    # DVE: f32 stt for taps 1,3
                    nc.vector.scalar_tensor_tensor(
                        mbuf[:, TCH:1024], p13[:, :TCH], b2s(dt, 1),
                        x_slice(dt, j0 + 1, TCH), op0=ALU.add, op1=ALU.mult)
                    nc.vector.scalar_tensor_tensor(
                        mbuf[:, 1024 + TCH:2048], p13[:, TCH:], b2s(dt, 3),
                        x_slice(dt, j0 + 3, TCH), op0=ALU.add, op1=ALU.mult)
                    # DVE: s-wide + fin for the previous iteration
                    emit_sw()
                    # ACT: silu + out DMA for the pair completed last iter
                    emit_silu()
                    if it % 2 == 0:
                        finbuf = wpool.tile([128, 1024], bf16, tag="fin")
                    sw_q.append((mbuf, finbuf, it % 2))
                    if it % 2 == 1:
                        silu_q.append((finbuf, it - 1))
                    # ACT: hT silu for chunk c+1, after this iter's pull
                    if hps_next is not None and dt == n_dt - 1:
                        mm1_silu(c + 1, hps_next)
            emit_sw()
            emit_silu(drain=True)
    nc.compile()
    return nc


def _prep_shards(x, w1, w2, b2, tok, d, h, halo, xstride):
    """Host-side shard prep. Returns list of per-core in_maps."""
    import ml_dtypes
    bf16 = ml_dtypes.bfloat16

    n_dt = d // 128
    n_hc = h // 128
    b, t, _ = x.shape
    shards_per_batch = (b * t // tok) // b
    w1_r = np.ascontiguousarray(
        w1.reshape(n_dt, 128, h).transpose(1, 0, 2)).astype(bf16)
    # w2 [h, d*W] -> [128, n_hc, n_dt, W, 128]
    w2_r = np.ascontiguousarray(
        w2.reshape(n_hc, 128, n_dt, 128, W)
        .transpose(1, 0, 2, 4, 3)).astype(bf16)
    b2_r = np.ascontiguousarray(
        b2.reshape(n_dt, 128, W).transpose(1, 0, 2)
        .reshape(128, n_dt * W)).astype(np.float32)

    in_maps = []
    for core in range(N_CORES):
        bi, half = divmod(core, shards_per_batch)
        t0 = half * tok
        xh = np.zeros((tok + halo, d), np.float32)
        lo = max(t0 - halo, 0)
        xh[halo - (t0 - lo):] = x[bi, lo: t0 + tok]
        xTc = np.zeros((128, n_dt, xstride), bf16)
        xTc[:, :, : tok + halo] = (
            xh.T.astype(bf16).reshape(n_dt, 128, tok + halo)
            .transpose(1, 0, 2))
        in_maps.append({
            "xT": xTc, "w1": w1_r, "w2r": w2_r, "b2r": b2_r})
    return in_maps


_NC_CACHE = {}


def kernel(x, w1, w2, b2, trace=False):
    from concourse.bass_utils import run_bass_kernel_spmd

    tok, d, h = TOK, D, H
    xstride = tok + HALO + 1  # even -> keeps bf16 4B alignment per dtile
    key = (tok, d, h)
    if key not in _NC_CACHE:
        _NC_CACHE[key] = _build_nc(tok, d, h, xstride=xstride)
    nc = _NC_CACHE[key]

    in_maps = _prep_shards(
        np.asarray(x, np.float32), np.asarray(w1, np.float32),
        np.asarray(w2, np.float32), np.asarray(b2, np.float32),
        tok, d, h, HALO, xstride)

    res = run_bass_kernel_spmd(nc, in_maps, core_ids=list(range(N_CORES)),
                               trace=trace)
    kernel.last_result = res

    n_dt = d // 128
    n_tc = tok // TCH
    shards_per_batch = (B * T // tok) // B
    out = np.empty((B, T, D), np.float32)
    for core in range(N_CORES):
        bi, half = divmod(core, shards_per_batch)
        oT = res.results[core]["outT"]  # [128, n_tc*n_dt*TCH]
        # [128p, c, dt, j] -> [c, j, dt, p] -> [tok, d]
        o = oT.reshape(128, n_tc, n_dt, TCH).transpose(1, 3, 2, 0)
        out[bi, half * tok:(half + 1) * tok] = (
            o.reshape(tok, d).astype(np.float32))
    return out
